# revision 3
# baseline (speedup 1.0000x reference)
"""Trainium2 Bass kernel for ExpressionAttentionLayer — pipelined rewrite.

Math per (batch b, head h), all on one core (core c -> batch c//2, heads
(c%2)*4..+4):
    k_fused = concat(K_gene, K_expr) @ Wk.T          [S, HD]
    q_fused = (concat(Q_gene, Q_expr) @ Wq.T) / 8    (scale folded into Wq)
    L       = q_fused @ k_fused.T                    [S, S]
    P       = exp(L)           (softmax numerator; |L| <~ 8)
    denom   = sum_k P          (full, pre-mask denominator)
    out     = (P * M[b]) @ V / denom[:, None]
    y       = out @ Wo.T       (bo added on host)

Device layout is "transposed": logits are computed as L^T[k, q] so the A@V
contraction (over k) and the denominator (ones-matmul over k, col-tiled to
run concurrently with A@V) stream from SBUF with no transposes.

Engine balance (the whole point of this rewrite): the four engines each get
~100us of work and the loop is software-pipelined so they overlap:
  - PE:     QK^T (2x row-packed K=64), A@V (M=64) || denominator (col-tiled)
  - ACT:    exp on most logit tiles
  - DVE:    mask multiply (f16 2x mode), Schraudolph bit-exp on some tiles,
            projection bias-copies, division drain
  - GPSIMD: every 3rd mask multiply
A@V for iteration i is emitted after QK^T/exp/mask of iteration i+1 so the
PE never stalls on the exp/mask round trip (pl is triple-buffered; the
baseline serialized here and ran ~2.5x slower).
"""

import os
import sys

import numpy as np

for _p in ("/opt/trn_rl_repo",):
    if os.path.isdir(_p) and _p not in sys.path:
        sys.path.insert(0, _p)

import concourse.bass as bass
import concourse.tile as tile
from concourse import bacc, mybir
from concourse.bass_utils import run_bass_kernel_spmd

B, S, H, HD = 4, 2048, 8, 64
D = H * HD
NCORES = 8
HPC = 4            # heads per core
KT = S // 128      # 16 k-chunks of 128
KP = KT // 2       # 8 k-chunk pairs
NQB = 4            # 512-wide q blocks
F16 = mybir.dt.float16
F32 = mybir.dt.float32
I16 = mybir.dt.int16
EXP = mybir.ActivationFunctionType.Exp
ADD = mybir.AluOpType.add
MULT = mybir.AluOpType.mult

# every Nth mask-multiply goes to GPSIMD instead of DVE (0 = never)
GPSIMD_EVERY = int(os.environ.get("K_GPSIMD_EVERY", "3"))
# every Nth exp is a Schraudolph bit-exp on DVE instead of ACT (0 = never).
# Off by default: HW-validated but the added ~3% per-tile exp error lands the
# worst per-core error at ~1.7e-2 vs the 2e-2 gate — too little margin.
BITEXP_EVERY = int(os.environ.get("K_BITEXP_EVERY", "0"))
# f16 Schraudolph constants: f16(int16(x*A + B)) ~= exp(x), max rel err 3.0%
EXP_A = 1477.3197218702985
EXP_B = 15315.75
# drain reciprocal: "safe" = plain reciprocal to f16. The custom-DVE
# reciprocal_approx_fast ("fast"/"fast16") produces garbage on HW through
# this execution path (uop tables not loaded) — do not enable.
DRAIN = os.environ.get("K_DRAIN", "safe")


def _bcast_part(ap, n):
    """Partition-broadcast view of a single-partition AP (stride-0, n rows)."""
    return bass.AP(tensor=ap.tensor, offset=ap.offset, ap=[[0, n]] + ap.ap[1:])


def _emit(nc, t):
    qcat, kcat, vex, mt, wk2, wq2, bk2, bq2, wo, yT = (
        t["qcat"], t["kcat"], t["vex"], t["mt"], t["wk2"], t["wq2"],
        t["bk2"], t["bq2"], t["wo"], t["yT"],
    )
    tc = t["tc"]
    ctx = t["ctx"]

    sing = ctx.enter_context(tc.tile_pool(name="sing", bufs=1))
    pin = ctx.enter_context(tc.tile_pool(name="pin", bufs=3))
    fused = ctx.enter_context(tc.tile_pool(name="fused", bufs=2))
    pexp = ctx.enter_context(tc.tile_pool(name="pexp", bufs=3))
    pmp = ctx.enter_context(tc.tile_pool(name="pmp", bufs=3))
    dr = ctx.enter_context(tc.tile_pool(name="dr", bufs=2))
    bcp = ctx.enter_context(tc.tile_pool(name="bcp", bufs=2))
    ypool = ctx.enter_context(tc.tile_pool(name="ypool", bufs=3))
    big = ctx.enter_context(tc.tile_pool(name="big", bufs=2, space="PSUM"))
    av = ctx.enter_context(tc.tile_pool(name="av", bufs=2, space="PSUM"))
    misc = ctx.enter_context(tc.tile_pool(name="misc", bufs=2, space="PSUM"))

    # ---- persistent SBUF state -------------------------------------------
    # DMA order = consumption order on a cold run: weights first (head-0
    # projections), then V, then the 8MB mask (first needed a few us in)
    wk_sb = sing.tile([128, 128], F16, tag="wk")
    wq_sb = sing.tile([128, 128], F16, tag="wq")
    nc.sync.dma_start(out=wk_sb[:], in_=wk2.ap())
    nc.sync.dma_start(out=wq_sb[:], in_=wq2.ap())
    bk_sb = sing.tile([128, 1], F32, tag="bk")
    bq_sb = sing.tile([128, 1], F32, tag="bq")
    nc.sync.dma_start(out=bk_sb[:], in_=bk2.ap())
    nc.sync.dma_start(out=bq_sb[:], in_=bq2.ap())
    v_sb = []
    for h in range(HPC):
        vt = sing.tile([128, KT * HD], F16, tag=f"v{h}", name=f"v{h}")
        nc.sync.dma_start(out=vt[:], in_=vex.ap()[h])
        v_sb.append(vt)
    mt_sb = sing.tile([128, KP * 4096], F16, tag="mt")
    for kp in range(KP):
        nc.sync.dma_start(out=mt_sb[:, kp * 4096:(kp + 1) * 4096], in_=mt.ap()[kp])
    wo_sb = sing.tile([128, 2 * D], F16, tag="wo")
    for c in range(2):
        nc.sync.dma_start(out=wo_sb[:, c * D:(c + 1) * D], in_=wo.ap()[c])
    ones_col = sing.tile([128, 1], F16, tag="ones_col")
    nc.vector.memset(ones_col[:], 1.0)
    ones_bc = sing.tile(
        [128, HD], F32 if DRAIN in ("fast", "mid") else F16, tag="ones_bc"
    )
    nc.vector.memset(ones_bc[:], 1.0)
    assert DRAIN in ("fast", "mid", "fast16", "lnexp", "safe"), DRAIN
    zeros_row = sing.tile([128, 128], F16, tag="zeros_row")
    nc.vector.memset(zeros_row[:], 0.0)
    attnT = [
        sing.tile([128, S], F16, tag=f"attnT{c}", name=f"attnT{c}") for c in range(2)
    ]

    st = {"git": 0, "pend": None}

    def emit_av(h, kp, qb, p_t, pm_t, avX):
        """A@V + denominator for one (k-pair, 512q) tile; the denominator
        ones-matmul is col-tiled (position db) so it streams concurrently
        with A@V."""
        eb = (h % 2) * 64
        db = 64 - eb
        k0, k1 = 2 * kp, 2 * kp + 1
        if kp == 0:
            # open the accumulator bank with a zeroing K=1 matmul: clears
            # has_written for the whole bank so the A@V (rows eb..eb+63) and
            # denominator (row db) writes below can all accumulate freely
            nc.tensor.matmul(
                avX[:, :], zeros_row[0:1, :], mt_sb[0:1, 0:512],
                start=True, stop=False, skip_group_check=True,
            )
        for ci, sl0 in ((k0, 0), (k1, 512)):
            last = kp == KP - 1 and ci == k1
            nc.tensor.matmul(
                avX[eb:eb + 64, :], v_sb[h][:, ci * HD:(ci + 1) * HD],
                pm_t[:, sl0:sl0 + 512],
                start=False, stop=last, tile_position=(0, eb),
                skip_group_check=True,
            )
            nc.tensor.matmul(
                avX[db:db + 1, :], ones_col[:],
                p_t[:, sl0:sl0 + 512],
                start=False, stop=last, tile_position=(0, db),
                skip_group_check=True,
            )

    def emit_drain(h, qb, avX):
        """attnT[.., qb block] = avX / denom  (late softmax division).
        1/denom [1, 512] is replicated across partitions with a K=1 matmul;
        the PSUM->SBUF copy of the replica alternates ACT/DVE."""
        eb = (h % 2) * 64
        db = 64 - eb
        chunk = h // 2
        q0 = qb * 512
        if DRAIN == "fast":
            rr = dr.tile([128, 512], F32, tag="rr")
            nc.vector.reciprocal_approx_fast(rr[db:db + 1, :], avX[db:db + 1, :])
        elif DRAIN == "mid":
            rr = dr.tile([128, 512], F32, tag="rr")
            nc.vector.reciprocal(rr[db:db + 1, :], avX[db:db + 1, :])
        elif DRAIN == "fast16":
            # approx reciprocal (fp32-only custom DVE op), then a cheap f16
            # downcast so the partition-broadcast matmul stays f16
            r32 = dr.tile([128, 512], F32, tag="t1")
            nc.vector.reciprocal_approx_fast(r32[db:db + 1, :], avX[db:db + 1, :])
            rr = dr.tile([128, 512], F16, tag="rr")
            nc.vector.tensor_copy(rr[db:db + 1, :], r32[db:db + 1, :])
        elif DRAIN == "lnexp":
            # 1/denom = exp(-ln(denom)) on ACT: both functions live in the
            # natural_log_exp_and_others table set, so no set switching
            t1 = dr.tile([128, 512], F32, tag="t1")
            nc.scalar.activation(
                out=t1[db:db + 1, :], in_=avX[db:db + 1, :],
                func=mybir.ActivationFunctionType.Ln,
            )
            rr = dr.tile([128, 512], F16, tag="rr")
            nc.scalar.activation(
                out=rr[db:db + 1, :], in_=t1[db:db + 1, :],
                func=EXP, scale=-1.0,
            )
        else:
            rr = dr.tile([128, 512], F16, tag="rr")
            nc.vector.reciprocal(rr[db:db + 1, :], avX[db:db + 1, :])
        pb = misc.tile([128, 512], F32, tag="pb")
        nc.tensor.matmul(
            pb[eb:eb + 64, :], ones_bc[db:db + 1, 0:64], rr[db:db + 1, :],
            start=True, stop=True, tile_position=(db, eb),
        )
        bc = bcp.tile([128, 512], F32, tag="bc")
        if (h * NQB + qb) % 2 == 0:
            nc.scalar.copy(bc[eb:eb + 64, :], pb[eb:eb + 64, :])
        else:
            nc.vector.tensor_copy(bc[eb:eb + 64, :], pb[eb:eb + 64, :])
        nc.vector.tensor_mul(
            attnT[chunk][eb:eb + 64, q0:q0 + 512],
            avX[eb:eb + 64, :], bc[eb:eb + 64, :],
        )

    # ---- per-head attention ----------------------------------------------
    for rep_h in range(HPC * t.get("repeats", 1)):
        h = rep_h % HPC

        qc = pin.tile([128, S], F16, tag="qc")
        kc = pin.tile([128, S], F16, tag="kc")
        nc.sync.dma_start(out=qc[:], in_=qcat.ap()[h])
        nc.sync.dma_start(out=kc[:], in_=kcat.ap()[h])

        # fused projections -> kf/qf in duplicated [128, S] d-major layout.
        # The PSUM->SBUF copy carries the bias (per-partition scalar) along.
        kf = fused.tile([128, S], F16, tag="kf")
        qf = fused.tile([128, S], F16, tag="qf")
        for src, w_sb, b_sb, dst, on_act in (
            (kc, wk_sb, bk_sb, kf, False), (qc, wq_sb, bq_sb, qf, True)
        ):
            for half in range(2):
                pp = big.tile([128, 1024], F32, tag="pl")
                for j in range(2):
                    o = half * 1024 + j * 512
                    nc.tensor.matmul(
                        pp[:, j * 512:(j + 1) * 512], w_sb[:], src[:, o:o + 512],
                        start=True, stop=True,
                    )
                dsl = dst[:, half * 1024:(half + 1) * 1024]
                if on_act:
                    nc.scalar.activation(
                        out=dsl, in_=pp[:],
                        func=mybir.ActivationFunctionType.Identity,
                        bias=b_sb[:, 0:1], scale=1.0,
                    )
                else:
                    nc.vector.tensor_scalar(dsl, pp[:], b_sb[:, 0:1], None, op0=ADD)

        avX = None
        for it in range(NQB * KP):
            qb, kp = divmod(it, KP)
            k0, k1 = 2 * kp, 2 * kp + 1
            q0 = qb * 512

            pl = big.tile([128, 1024], F32, tag="pl")
            nc.tensor.matmul(
                pl[:, 0:512], kf[0:64, k0 * 128:(k0 + 1) * 128],
                qf[0:64, q0:q0 + 512],
                start=True, stop=True, tile_position=(0, 0),
            )
            nc.tensor.matmul(
                pl[:, 512:1024], kf[64:128, k1 * 128:(k1 + 1) * 128],
                qf[64:128, q0:q0 + 512],
                start=True, stop=True, tile_position=(64, 0),
            )

            p_t = pexp.tile([128, 1024], F16, tag="p")
            git = st["git"]
            st["git"] = git + 1
            if BITEXP_EVERY and git % BITEXP_EVERY == BITEXP_EVERY - 1:
                nc.vector.tensor_scalar(
                    p_t[:].bitcast(I16), pl[:], EXP_A, EXP_B, op0=MULT, op1=ADD
                )
            else:
                nc.scalar.activation(out=p_t[:], in_=pl[:], func=EXP)

            pm_t = pmp.tile([128, 1024], F16, tag="pm")
            mul_eng = (
                nc.gpsimd
                if GPSIMD_EVERY and (git % GPSIMD_EVERY == GPSIMD_EVERY - 1)
                else nc.vector
            )
            mul_eng.tensor_mul(
                pm_t[:], p_t[:],
                mt_sb[:, kp * 4096 + qb * 1024: kp * 4096 + qb * 1024 + 1024],
            )

            if kp == 0:
                avX = av.tile([128, 512], F32, tag="avX")

            # software pipeline: A@V trails QK by one iteration so the PE
            # never waits for the exp/mask round trip
            if st["pend"] is not None:
                emit_av(*st["pend"])
                ph, pkp, pqb, _, _, pavX = st["pend"]
                if pkp == KP - 1:
                    emit_drain(ph, pqb, pavX)
            st["pend"] = (h, kp, qb, p_t, pm_t, avX)

    if st["pend"] is not None:
        emit_av(*st["pend"])
        ph, pkp, pqb, _, _, pavX = st["pend"]
        emit_drain(ph, pqb, pavX)
        st["pend"] = None

    # ---- partial out_proj: yT[do, s] = sum_di Wo_slice[do, di] attnT[di, s]
    for rep_o in range(t.get("repeats", 1)):
        for st_i in range(S // 512):
            for do_i in range(D // 128):
                idx = st_i * (D // 128) + do_i
                py = av.tile([128, 512], F32, tag="avX")
                for c in range(2):
                    nc.tensor.matmul(
                        py[:], wo_sb[:, c * D + do_i * 128: c * D + (do_i + 1) * 128],
                        attnT[c][:, st_i * 512:(st_i + 1) * 512],
                        start=(c == 0), stop=(c == 1),
                    )
                yt = ypool.tile([128, 512], F16, tag="y")
                nc.scalar.copy(yt[:], py[:])
                nc.sync.dma_start(
                    out=yT.ap()[do_i * 128:(do_i + 1) * 128,
                                st_i * 512:(st_i + 1) * 512],
                    in_=yt[:],
                )


_NC_CACHE = None


def build_program(repeats=1):
    global _NC_CACHE
    if _NC_CACHE is not None and repeats == 1:
        return _NC_CACHE
    from contextlib import ExitStack

    nc = bacc.Bacc("TRN2", target_bir_lowering=False, debug=False, num_devices=NCORES)
    t = {
        "qcat": nc.dram_tensor("qcat", [HPC, 128, S], F16, kind="ExternalInput"),
        "kcat": nc.dram_tensor("kcat", [HPC, 128, S], F16, kind="ExternalInput"),
        "vex": nc.dram_tensor("vex", [HPC, 128, KT * HD], F16, kind="ExternalInput"),
        "mt": nc.dram_tensor("mt", [KP, 128, 4096], F16, kind="ExternalInput"),
        "wk2": nc.dram_tensor("wk2", [128, 128], F16, kind="ExternalInput"),
        "wq2": nc.dram_tensor("wq2", [128, 128], F16, kind="ExternalInput"),
        "bk2": nc.dram_tensor("bk2", [128, 1], F32, kind="ExternalInput"),
        "bq2": nc.dram_tensor("bq2", [128, 1], F32, kind="ExternalInput"),
        "wo": nc.dram_tensor("wo", [2, 128, D], F16, kind="ExternalInput"),
        "yT": nc.dram_tensor("yT", [D, S], F16, kind="ExternalOutput"),
    }
    with tile.TileContext(nc) as tc, nc.allow_low_precision(
        reason="fp16 attention core"
    ):
        with ExitStack() as ctx:
            t["tc"] = tc
            t["ctx"] = ctx
            t["repeats"] = repeats
            _emit(nc, t)
    nc.compile()
    if repeats == 1:
        _NC_CACHE = nc
    return nc


def make_in_maps(Q_gene, K_gene, Q_expr, K_expr, V_expr, M, Wk, bk, Wq, bq, Wo, bo):
    """Host-side sharding + layout prep (fp16 conversion, transposes)."""
    f32 = np.float32
    f16 = np.float16
    scale = 1.0 / np.sqrt(HD)
    wk2 = np.ascontiguousarray(
        np.concatenate([np.asarray(Wk, f32).T] * 2, axis=1)
    ).astype(f16)
    wq2 = np.ascontiguousarray(
        np.concatenate([np.asarray(Wq, f32).T * scale] * 2, axis=1)
    ).astype(f16)
    bk2 = np.concatenate([np.asarray(bk, f32)] * 2).reshape(128, 1)
    bq2 = (np.concatenate([np.asarray(bq, f32)] * 2) * scale).reshape(128, 1)

    per_batch = []
    for b in range(B):
        MTb = np.asarray(M[b], f32).T.astype(f16)            # [k, q]
        # slot (kp, qb) holds [k-chunk 2kp: 512 q | k-chunk 2kp+1: 512 q]
        mt_host = np.ascontiguousarray(
            MTb.reshape(KP, 2, 128, NQB, 512).transpose(0, 2, 3, 1, 4)
        ).reshape(KP, 128, 4096)
        qg = np.asarray(Q_gene[b], f32).transpose(1, 2, 0)   # [H, HD, S]
        qe = np.asarray(Q_expr[b], f32).transpose(1, 2, 0)
        kg = np.asarray(K_gene[b], f32).transpose(1, 2, 0)
        ke = np.asarray(K_expr[b], f32).transpose(1, 2, 0)
        vv = np.asarray(V_expr[b], f32).transpose(1, 0, 2)   # [H, S, HD]
        per_batch.append((mt_host, qg, qe, kg, ke, vv))

    in_maps = []
    for c in range(NCORES):
        b = c // 2
        h0 = (c % 2) * HPC
        mt_host, qg, qe, kg, ke, vv = per_batch[b]
        qcat = np.concatenate([qg[h0:h0 + HPC], qe[h0:h0 + HPC]], axis=1).astype(f16)
        kcat = np.concatenate([kg[h0:h0 + HPC], ke[h0:h0 + HPC]], axis=1).astype(f16)
        vex = np.ascontiguousarray(
            vv[h0:h0 + HPC]
            .reshape(HPC, KT, 128, HD)
            .transpose(0, 2, 1, 3)
            .reshape(HPC, 128, KT * HD)
        ).astype(f16)
        wo_dev = np.ascontiguousarray(
            np.asarray(Wo, f32)[:, h0 * HD:(h0 + HPC) * HD].T.reshape(2, 128, D)
        ).astype(f16)
        in_maps.append(
            {
                "qcat": np.ascontiguousarray(qcat),
                "kcat": np.ascontiguousarray(kcat),
                "vex": vex,
                "mt": mt_host,
                "wk2": wk2,
                "wq2": wq2,
                "bk2": bk2,
                "bq2": bq2,
                "wo": wo_dev,
            }
        )
    return in_maps


def assemble_output(results, bo):
    out = np.empty((B, S, D), np.float32)
    bo = np.asarray(bo, np.float32)
    for b in range(B):
        yt = results[2 * b]["yT"].astype(np.float32) + results[2 * b + 1][
            "yT"
        ].astype(np.float32)
        out[b] = yt.T + bo[None, :]
    return out


def kernel(**inputs):
    nc = build_program()
    in_maps = make_in_maps(**inputs)
    res = run_bass_kernel_spmd(nc, in_maps, list(range(NCORES))).results
    return assemble_output(res, inputs["bo"])
